# revision 1
# baseline (speedup 1.0000x reference)
"""Trainium2 Bass kernel for nn_AttentionCircuit (neuron-mixture attention).

Self-contained: accepts FULL inputs, shards across 8 NeuronCores, runs a
Bass/Tile SPMD kernel, gathers the full output.

Sharding: core c = (b, g) with b = c // 4 (batch), g = c % 4 (head-group of
4 heads = 256 channels).  Features are sequence-split within each batch
group and all-gathered; restore + attention are head-group-parallel; the
output projection uses a column shard of W_O after all-gathering the
attention output (transposed layout).  All TensorEngine compute in bf16,
f32 I/O and PSUM accumulation.
"""

import sys

for _p in ("/opt/trn_rl_repo",):
    if _p not in sys.path:
        sys.path.append(_p)

import numpy as np
from dataclasses import dataclass

import concourse.bass as bass
import concourse.bacc as bacc
import concourse.mybir as mybir
import concourse.tile as tile
from concourse import masks
from concourse.bass_utils import run_bass_kernel_spmd

try:
    import ml_dtypes

    BF16 = ml_dtypes.bfloat16
except ImportError:  # pragma: no cover
    BF16 = np.float32


def _install_neff_disk_cache():
    """Cache walrus BIR->NEFF compiles on disk (keyed by BIR bytes) so
    repeated runs of the identical graph skip the multi-minute compile."""
    import hashlib, os, tempfile
    from concourse import bass2jax

    if getattr(bass2jax, "_ant_neff_cache_installed", False):
        return
    orig = bass2jax.compile_bir_kernel
    cache_dir = os.path.join(tempfile.gettempdir(), "bass_neff_cache")
    os.makedirs(cache_dir, exist_ok=True)

    def cached(bir_json, tmpdir, neff_name="file.neff"):
        key = hashlib.sha256(bir_json).hexdigest()
        path = os.path.join(cache_dir, key + ".neff")
        dst = os.path.join(tmpdir, neff_name)
        if os.path.exists(path):
            import shutil

            shutil.copy(path, dst)
            return dst
        neff = orig(bir_json, tmpdir, neff_name=neff_name)
        try:
            import shutil

            shutil.copy(neff, path)
        except OSError:
            pass
        return neff

    bass2jax.compile_bir_kernel = cached
    bass2jax._ant_neff_cache_installed = True


_install_neff_disk_cache()

F32 = mybir.dt.float32
BF = mybir.dt.bfloat16
P = 128  # partitions


@dataclass(frozen=True)
class Cfg:
    B: int = 2
    S: int = 2048
    D: int = 1024
    R: int = 128
    N: int = 8
    H: int = 16
    cores: int = 8

    @property
    def G(self):  # cores per batch == head groups
        return self.cores // self.B

    @property
    def S_sl(self):  # sequence slice per core (feature stage)
        return self.S // self.G

    @property
    def COLS(self):  # channel columns per core
        return self.D // self.G

    @property
    def Hpc(self):  # heads per core
        return self.H // self.G

    @property
    def dh(self):
        return self.D // self.H

    @property
    def KD(self):  # k-tiles over D
        return self.D // P

    @property
    def NR(self):
        return self.N * self.R

    @property
    def KNR(self):  # k-tiles over N*R
        return self.NR // P

    @property
    def ST(self):  # s-tiles over full S
        return self.S // P

    @property
    def ST_sl(self):  # s-tiles over S slice
        return self.S_sl // P

    @property
    def CT(self):  # 128-col tiles over COLS
        return (self.COLS + P - 1) // P


FULL = Cfg()


def _ceil_div(a, b):
    return (a + b - 1) // b


def build_nc(cfg: Cfg = FULL, fake_cc: bool = False) -> bacc.Bacc:
    """Build + compile the SPMD graph (identical on every core).

    fake_cc=True replaces collectives with local DMA replication (wrong
    results) so the single-core TimelineSim can cost-model the kernel.
    """
    c = cfg
    assert c.R == P and c.D % P == 0 and c.S_sl % P == 0
    assert P % c.dh == 0 and c.COLS % c.dh == 0 and c.COLS % P == 0

    nc = bacc.Bacc(
        "TRN2",
        target_bir_lowering=False,
        debug=False,
        num_devices=1 if fake_cc else c.cores,
    )

    def all_gather(in_t, out_t):
        if fake_cc:
            for g in range(c.G):
                nc.sync.dma_start(out=out_t.ap()[g], in_=in_t.ap()[:])
        else:
            nc.gpsimd.collective_compute(
                "AllGather",
                mybir.AluOpType.bypass,
                replica_groups=rgroups,
                ins=[in_t.ap().opt()],
                outs=[out_t.ap().opt()],
            )

    # ---- DRAM parameters (host-prepped layouts, see shard_inputs) ----
    xT = nc.dram_tensor("xT", [P, c.KD, c.S_sl], BF, kind="ExternalInput")
    fqk = nc.dram_tensor("fqk", [P, c.KD, c.NR], BF, kind="ExternalInput")
    fv = nc.dram_tensor("fv", [P, c.KD, c.NR], BF, kind="ExternalInput")
    rqk = nc.dram_tensor("rqk", [P, c.KNR, c.COLS], BF, kind="ExternalInput")
    rv = nc.dram_tensor("rv", [P, c.KNR, c.COLS], BF, kind="ExternalInput")
    wo = nc.dram_tensor("wo", [P, c.KD, c.COLS], BF, kind="ExternalInput")
    # combine scalars (feature weights for this core's s-slice), f32
    wcomb = nc.dram_tensor("wcomb", [P, 3, c.ST_sl, c.N], F32, kind="ExternalInput")
    # restore weights, full S, bf16 (broadcast source): [3, N, S]
    wsm = nc.dram_tensor("wsm", [3 * c.N, c.S], BF, kind="ExternalInput")
    out_d = nc.dram_tensor("out", [c.S, c.COLS], F32, kind="ExternalOutput")

    group0 = list(range(c.G))
    group1 = list(range(c.G, 2 * c.G))
    rgroups = [group0, group1]

    scale = 1.0 / float(np.sqrt(c.dh))
    DHO = c.dh + 1  # dh + ones column

    from contextlib import ExitStack

    with tile.TileContext(nc) as tc, ExitStack() as stack:
        # ------- constants -------
        const_pool = stack.enter_context(tc.tile_pool(name="const", bufs=1))
        ident = const_pool.tile([P, P], BF)
        masks.make_identity(nc, ident[:])
        cmask = const_pool.tile([P, P], BF)
        masks.make_upper_triangular(nc, cmask[:], val=1.0, diag=True)

        # ------- long-lived SBUF residents (DMAs for stage-C/F consumers are
        # emitted after stage A so they don't delay the critical xT/f loads)
        res_pool = stack.enter_context(tc.tile_pool(name="residents", bufs=1))
        rqk_sb = res_pool.tile([P, c.KNR, c.COLS], BF)
        rv_sb = res_pool.tile([P, c.KNR, c.COLS], BF)
        wo_sb = res_pool.tile([P, c.KD, c.COLS], BF)
        wcomb_sb = res_pool.tile([P, 3, c.ST_sl, c.N], F32)
        nc.sync.dma_start(out=wcomb_sb[:], in_=wcomb[:])

        qT_sb = res_pool.tile([P, c.CT, c.S], BF)
        kT_sb = res_pool.tile([P, c.CT, c.S], BF)
        v_sb = res_pool.tile([P, c.ST, c.Hpc * DHO], BF)
        attn_sb = res_pool.tile([P, c.ST, c.Hpc * c.dh], BF)

        # ones columns of v_sb
        v4 = v_sb[:].rearrange("p st (h x) -> p st h x", x=DHO)
        nc.gpsimd.memset(v4[:, :, :, c.dh : c.dh + 1], 1.0)

        # DRAM bounce buffers for collectives (aT gathered per column tile so
        # the first collective overlaps attention of the remaining heads)
        hT_in = nc.dram_tensor("hT_in", [3, P, c.S_sl], BF)
        hT_out = nc.dram_tensor("hT_out", [c.G, 3, P, c.S_sl], BF)
        aT_in_l = [
            nc.dram_tensor(f"aT_in{ct}", [P, c.S], BF) for ct in range(c.CT)
        ]
        aT_out_l = [
            nc.dram_tensor(f"aT_out{ct}", [c.G, P, c.S], BF) for ct in range(c.CT)
        ]

        # wrep/g pools span stages A+C: the broadcasts are emitted during
        # stage A so they complete on the DMA queues before restore needs them
        cstack = ExitStack()
        wrep_pool = cstack.enter_context(
            tc.tile_pool(name="wrep", bufs=c.N + 2)
        )
        wr_tiles = {}
        g_tiles = {}

        # ================= Stage A: features on the s-slice =================
        with (
            tc.tile_pool(name="featA", bufs=2) as fpool,
            tc.tile_pool(name="featP", bufs=6, space="PSUM") as fps_pool,
            tc.tile_pool(name="featH", bufs=2) as hpool,
            tc.tile_pool(name="featHT", bufs=2, space="PSUM") as htps_pool,
        ):
            xT_sb = fpool.tile([P, c.KD, c.S_sl], BF, tag="xT", bufs=1)
            fqk_sb = fpool.tile([P, c.KD, c.NR], BF, tag="fqk", bufs=1)
            fv_sb = fpool.tile([P, c.KD, c.NR], BF, tag="fv", bufs=1)
            for k in range(c.KD):
                nc.sync.dma_start(out=xT_sb[:, k, :], in_=xT[:, k, :])
                nc.sync.dma_start(out=fqk_sb[:, k, :], in_=fqk[:, k, :])
                nc.sync.dma_start(out=fv_sb[:, k, :], in_=fv[:, k, :])

            # W_rep broadcasts: V's go through the idle Pool engine during
            # stage A (keeps the DMA queues clear); q/k replicate via DMA
            # during stage C when the queues have slack.
            def make_wr(t, use_pool=False):
                for n in range(c.N):
                    row = t * c.N + n
                    wr = wrep_pool.tile([P, c.S], BF, tag="wrep", name=f"wr_{row}")
                    if use_pool:
                        wst = wrep_pool.tile(
                            [1, c.S], BF, tag="wstage", bufs=2, name=f"wst_{row}"
                        )
                        nc.sync.dma_start(out=wst[:], in_=wsm[row : row + 1, :])
                        nc.gpsimd.partition_broadcast(wr[:], wst[0:1, :])
                    else:
                        nc.sync.dma_start(
                            out=wr[:],
                            in_=wsm.ap()[row : row + 1, :].broadcast_to([P, c.S]),
                        )
                    wr_tiles[(t, n)] = wr

            make_wr(2, use_pool=True)

            f_chunk = min(c.NR, 512)
            n_ch = _ceil_div(c.NR, f_chunk)
            n_per_ch = f_chunk // c.R
            for st in range(c.ST_sl):
                ps_tiles = {}
                for pi, f_sb in ((0, fqk_sb), (1, fv_sb)):
                    for ch in range(n_ch):
                        ps = fps_pool.tile([P, f_chunk], F32, tag="feat")
                        ps_tiles[(pi, ch)] = ps
                        lo = f_chunk * ch
                        hi = min(c.NR, lo + f_chunk)
                        for k in range(c.KD):
                            nc.tensor.matmul(
                                ps[:, 0 : hi - lo],
                                lhsT=xT_sb[:, k, P * st : P * (st + 1)],
                                rhs=f_sb[:, k, lo:hi],
                                start=(k == 0),
                                stop=(k == c.KD - 1),
                            )
                # copy all_h PSUM -> SBUF bf16 once (cheap), then combine in
                # 2-byte SBUF mode: h[s, r] = sum_n w[s, n] * all_h[s, n*R+r]
                ah_tiles = {}
                for pi in (0, 1):
                    for ch in range(n_ch):
                        ah = hpool.tile([P, f_chunk], BF, tag="ah", bufs=4)
                        nc.scalar.copy(ah[:], ps_tiles[(pi, ch)][:])
                        ah_tiles[(pi, ch)] = ah
                for t, pi in ((0, 0), (1, 0), (2, 1)):
                    h_t = hpool.tile([P, c.R], BF, tag="hacc")
                    for n in range(c.N):
                        ah = ah_tiles[(pi, n // n_per_ch)]
                        src = ah[:, c.R * (n % n_per_ch) : c.R * (n % n_per_ch + 1)]
                        if n == 0:
                            nc.vector.tensor_scalar(
                                out=h_t[:],
                                in0=src,
                                scalar1=wcomb_sb[:, t, st, 0:1],
                                scalar2=None,
                                op0=mybir.AluOpType.mult,
                            )
                        else:
                            nc.vector.scalar_tensor_tensor(
                                out=h_t[:],
                                in0=src,
                                scalar=wcomb_sb[:, t, st, n : n + 1],
                                in1=h_t[:],
                                op0=mybir.AluOpType.mult,
                                op1=mybir.AluOpType.add,
                            )
                    htp = htps_pool.tile([P, P], BF, tag="htp")
                    nc.tensor.transpose(htp[:], h_t[:], ident[:])
                    hT_sl = hpool.tile([P, P], BF, tag="hT", bufs=3)
                    nc.scalar.copy(hT_sl[:], htp[:, :])
                    nc.sync.dma_start(
                        out=hT_in[t, :, P * st : P * (st + 1)], in_=hT_sl[:]
                    )

            # ---- AllGather h^T across the batch group ----
            all_gather(hT_in, hT_out)

        # deferred resident loads (consumed by stage C/F)
        nc.sync.dma_start(out=rv_sb[:], in_=rv[:])
        nc.sync.dma_start(out=rqk_sb[:], in_=rqk[:])
        nc.sync.dma_start(out=wo_sb[:], in_=wo[:])

        # hT_full[r, t, g, s_in]  (s blocked by source rank g), per-block DMAs
        hT_sb = res_pool.tile([P, 3, c.G, c.S_sl], BF)
        for t in range(3):
            for g in range(c.G):
                nc.sync.dma_start(
                    out=hT_sb[:, t, g, :], in_=hT_out.ap()[g, t, :, :]
                )

        # ============ Stage C: restore projections (V, then Q^T/K^T) ============
        # g tiles are per-(tensor, n); the multiplies are chunked per source
        # block and split across DVE and GpSimd so the PE can start each
        # accumulation as soon as possible.
        g_pool = cstack.enter_context(tc.tile_pool(name="g", bufs=2 * c.N + 2))
        n_sch = _ceil_div(c.S, 512)

        def make_g(t):
            for n in range(c.N):
                row = t * c.N + n
                g_t = g_pool.tile([P, c.S], BF, tag="g", name=f"g_{row}")
                g_tiles[(t, n)] = g_t
            for blk in range(c.G):
                lo, hi = c.S_sl * blk, c.S_sl * (blk + 1)
                for n in range(c.N):
                    eng = nc.vector
                    eng.tensor_mul(
                        g_tiles[(t, n)][:, lo:hi],
                        hT_sb[:, t, blk, :],
                        wr_tiles[(t, n)][:, lo:hi],
                    )

        # ---- V ----
        make_g(2)
        with tc.tile_pool(name="vps", bufs=4, space="PSUM") as vps_pool:
            for st in range(c.ST):
                vps = vps_pool.tile([P, c.COLS], F32, tag="vps")
                for n in range(c.KNR):
                    nc.tensor.matmul(
                        vps[:, :],
                        lhsT=g_tiles[(2, n)][:, P * st : P * (st + 1)],
                        rhs=rv_sb[:, n, :],
                        start=(n == 0),
                        stop=(n == c.KNR - 1),
                    )
                # scatter into per-head blocks of v_sb (stride dh+1)
                nc.scalar.copy(
                    v4[:, st, :, 0 : c.dh],
                    vps[:, :].rearrange("p (h x) -> p h x", x=c.dh),
                )

        # ---- Q^T / K^T, column tile ct=0 first, then ct=1 ----
        make_wr(0)
        make_g(0)
        make_wr(1)
        make_g(1)
        with tc.tile_pool(name="rps", bufs=2, space="PSUM") as rps_pool:
            for ct in range(c.CT):
                for t, dst, r_sb in ((0, qT_sb, rqk_sb), (1, kT_sb, rqk_sb)):
                    pt = min(P, c.COLS - P * ct)
                    rps = rps_pool.tile([P, c.S], F32, tag="rps")
                    for ch in range(n_sch):
                        lo, hi = 512 * ch, min(c.S, 512 * ch + 512)
                        for n in range(c.KNR):
                            nc.tensor.matmul(
                                rps[:pt, lo:hi],
                                lhsT=r_sb[:, n, P * ct : P * ct + pt],
                                rhs=g_tiles[(t, n)][:, lo:hi],
                                start=(n == 0),
                                stop=(n == c.KNR - 1),
                            )
                    nc.scalar.copy(dst[:pt, ct, :], rps[:pt, :])
        cstack.close()

        # ================= Stage D: causal attention per head =================
        with (
            tc.tile_pool(name="probs", bufs=c.ST + 2) as pr_pool,
            tc.tile_pool(name="sps", bufs=3, space="PSUM") as sps_pool,
            tc.tile_pool(name="avps", bufs=1, space="PSUM") as av_pool,
            tc.tile_pool(name="attn_small", bufs=4) as asm_pool,
            tc.tile_pool(name="atps", bufs=1, space="PSUM") as atps_pool,
        ):
            for h in range(c.Hpc):
                ct = (c.dh * h) // P
                off = (c.dh * h) % P
                probs = []
                for j in range(c.ST):
                    qlo = P * j
                    qn = c.S - qlo
                    pj = pr_pool.tile([P, c.S], BF, tag="probs")
                    probs.append(pj)
                    SCH = 1024  # scores chunk (2 PSUM banks); exp whole chunk
                    for ch in range(_ceil_div(qn, SCH)):
                        lo = qlo + SCH * ch
                        hi = min(c.S, lo + SCH)
                        sps = sps_pool.tile([P, SCH], F32, tag="sps")
                        for sub in range(_ceil_div(hi - lo, 512)):
                            slo, shi = lo + 512 * sub, min(hi, lo + 512 * sub + 512)
                            nc.tensor.matmul(
                                sps[:, slo - lo : shi - lo],
                                lhsT=kT_sb[off : off + c.dh, ct, qlo : qlo + P],
                                rhs=qT_sb[off : off + c.dh, ct, slo:shi],
                                start=True,
                                stop=True,
                            )
                        nc.scalar.activation(
                            pj[:, lo - qlo : hi - qlo],
                            sps[:, 0 : hi - lo],
                            mybir.ActivationFunctionType.Exp,
                            scale=scale,
                        )
                    # mask the diagonal tile (keep q >= k)
                    nc.vector.tensor_mul(pj[:, 0:P], pj[:, 0:P], cmask[:])
                    # AV for q-tile j: k-tiles 0..j
                    av = av_pool.tile([P, DHO], F32, tag="av")
                    for j2 in range(j + 1):
                        nc.tensor.matmul(
                            av[:, :],
                            lhsT=probs[j2][:, P * (j - j2) : P * (j - j2) + P],
                            rhs=v_sb[:, j2, DHO * h : DHO * (h + 1)],
                            start=(j2 == 0),
                            stop=(j2 == j),
                        )
                    rec = asm_pool.tile([P, 1], F32, tag="rec")
                    nc.vector.reciprocal(rec[:], av[:, c.dh : c.dh + 1])
                    nc.vector.tensor_scalar(
                        out=attn_sb[:, j, c.dh * h : c.dh * (h + 1)],
                        in0=av[:, 0 : c.dh],
                        scalar1=rec[:],
                        scalar2=None,
                        op0=mybir.AluOpType.mult,
                    )

                # once both heads of a column tile are done: transpose that
                # tile, ship it, and launch its all-gather (overlaps with the
                # remaining heads' attention)
                if (h + 1) * c.dh % P == 0:
                    ct2 = ((h + 1) * c.dh) // P - 1
                    for st in range(c.ST):
                        atp = atps_pool.tile([P, P], BF, tag="atp")
                        nc.tensor.transpose(
                            atp[:, :],
                            attn_sb[:, st, P * ct2 : P * (ct2 + 1)],
                            ident[:],
                        )
                        at_sl = asm_pool.tile([P, P], BF, tag="at_sl")
                        nc.vector.tensor_copy(at_sl[:, :], atp[:, :])
                        nc.sync.dma_start(
                            out=aT_in_l[ct2][:, P * st : P * (st + 1)],
                            in_=at_sl[:, :],
                        )
                    all_gather(aT_in_l[ct2], aT_out_l[ct2])

        # ================= Stage F: output projection =================
        # k-outer accumulation in arrival order (ct-major) so matmuls start
        # as soon as the first gathered column tile lands.
        with (
            tc.tile_pool(name="aT_full", bufs=1) as atf_pool,
            tc.tile_pool(name="ops", bufs=8, space="PSUM") as ops_pool,
            tc.tile_pool(name="osb", bufs=4) as osb_pool,
        ):
            aTf_sb = atf_pool.tile([P, c.G * c.CT, c.S], BF)
            arrival = []  # kd indices in DMA order
            for ct in range(c.CT):
                for g in range(c.G):
                    kd = g * c.CT + ct
                    arrival.append(kd)
                    nc.sync.dma_start(
                        out=aTf_sb[:, kd, :], in_=aT_out_l[ct].ap()[g]
                    )
            kt_tot = c.G * c.CT  # == KD when COLS*G == D
            GRP = 4  # st-tiles per pass (PSUM banks)
            for grp in range(_ceil_div(c.ST, GRP)):
                sts = range(GRP * grp, min(c.ST, GRP * (grp + 1)))
                ops_t = {
                    st: ops_pool.tile([P, c.COLS], F32, tag="ops", name=f"ops_{st}")
                    for st in sts
                }
                for ki, kd in enumerate(arrival):
                    for st in sts:
                        nc.tensor.matmul(
                            ops_t[st][:, :],
                            lhsT=aTf_sb[:, kd, P * st : P * (st + 1)],
                            rhs=wo_sb[:, kd, :],
                            start=(ki == 0),
                            stop=(ki == kt_tot - 1),
                        )
                for st in sts:
                    osb = osb_pool.tile([P, c.COLS], F32, tag="osb")
                    nc.scalar.copy(osb[:], ops_t[st][:, :])
                    nc.sync.dma_start(
                        out=out_d.ap()[P * st : P * (st + 1), :], in_=osb[:]
                    )

    nc.compile()
    return nc


# ---------------------------------------------------------------------------
# Host-side sharding / gathering
# ---------------------------------------------------------------------------


def shard_inputs(
    inputs: dict,
    cfg: Cfg = FULL,
) -> list[dict]:
    c = cfg
    x = np.asarray(inputs["x"], np.float32)
    fqk_n = np.asarray(inputs["f_qk_neurons"], np.float32)
    fv_n = np.asarray(inputs["f_v_neurons"], np.float32)
    rqk_n = np.asarray(inputs["r_qk_neurons"], np.float32)
    rv_n = np.asarray(inputs["r_v_neurons"], np.float32)
    w_o = np.asarray(inputs["W_O"], np.float32)

    def tile_p(a, kt):  # [D, M] -> [P, kt, M]
        d, m = a.shape
        assert d == kt * P
        return np.ascontiguousarray(a.reshape(kt, P, m).transpose(1, 0, 2))

    # [N, D, R] -> [D, N*R]
    f_qk_flat = fqk_n.transpose(1, 0, 2).reshape(c.D, c.NR)
    f_v_flat = fv_n.transpose(1, 0, 2).reshape(c.D, c.NR)
    # [N, R, D] -> [N*R, D]
    r_qk_flat = rqk_n.reshape(c.NR, c.D)
    r_v_flat = rv_n.reshape(c.NR, c.D)

    in_maps = []
    for core in range(c.cores):
        b, g = core // c.G, core % c.G
        sl = slice(c.S_sl * g, c.S_sl * (g + 1))
        cols = slice(c.COLS * g, c.COLS * (g + 1))

        xT = x[b].T[:, sl]  # [D, S_sl]

        wq = np.asarray(inputs["fqk_weights_Q"], np.float32)[b, sl]  # [S_sl, N]
        wk = np.asarray(inputs["fqk_weights_K"], np.float32)[b, sl]
        wv = np.asarray(inputs["fv_weights"], np.float32)[b, sl]
        wcomb = np.stack([wq, wk, wv], 0)  # [3, S_sl, N]
        wcomb = np.ascontiguousarray(
            wcomb.reshape(3, c.ST_sl, P, c.N).transpose(2, 0, 1, 3)
        )  # [P, 3, ST_sl, N]

        wsm = np.stack(
            [
                np.asarray(inputs["rqk_weights_Q"], np.float32)[b].T,
                np.asarray(inputs["rqk_weights_K"], np.float32)[b].T,
                np.asarray(inputs["rv_weights"], np.float32)[b].T,
            ],
            0,
        ).reshape(3 * c.N, c.S)  # [3N, S]

        m = {
            "xT": tile_p(xT, c.KD).astype(BF16),
            "fqk": tile_p(f_qk_flat, c.KD).astype(BF16),
            "fv": tile_p(f_v_flat, c.KD).astype(BF16),
            "rqk": tile_p(r_qk_flat[:, cols], c.KNR).astype(BF16),
            "rv": tile_p(r_v_flat[:, cols], c.KNR).astype(BF16),
            "wo": tile_p(w_o[:, cols], c.KD).astype(BF16),
            "wcomb": wcomb.astype(np.float32),
            "wsm": wsm.astype(BF16),
        }
        in_maps.append(m)
    return in_maps


def gather_output(results: list[dict], cfg: Cfg = FULL) -> np.ndarray:
    c = cfg
    out = np.empty((c.B, c.S, c.D), np.float32)
    for core in range(c.cores):
        b, g = core // c.G, core % c.G
        out[b, :, c.COLS * g : c.COLS * (g + 1)] = np.asarray(
            results[core]["out"], np.float32
        )
    return out


_NC_CACHE = {}


def get_nc(cfg: Cfg = FULL) -> bacc.Bacc:
    if cfg not in _NC_CACHE:
        _NC_CACHE[cfg] = build_nc(cfg)
    return _NC_CACHE[cfg]


def kernel(**inputs) -> np.ndarray:
    cfg = FULL
    nc = get_nc(cfg)
    in_maps = shard_inputs(inputs, cfg)
    res = run_bass_kernel_spmd(nc, in_maps, core_ids=list(range(cfg.cores)))
    return gather_output(res.results, cfg)



# revision 76
# speedup vs baseline: 1.1894x; 1.1894x over previous
"""Trainium2 Bass kernel for nn_AttentionCircuit (neuron-mixture attention).

Self-contained: accepts FULL inputs, shards across 8 NeuronCores, runs a
Bass/Tile SPMD kernel, gathers the full output.

Sharding: core c = (b, g) with b = c // 4 (batch), g = c % 4 (head-group of
4 heads = 256 channels).  Features are sequence-split within each batch
group and all-gathered (chunked, overlapped with feature compute); restore +
attention are head-group-parallel; the output projection is ROW-sharded
(each core contracts its own 256 attention channels against W_O rows) and
finished with a ReduceScatter that leaves each core its 256-column shard.
All TensorEngine compute in bf16, f32 I/O; scores kept in bf16 PSUM so the
softmax exp is one Activation instruction per q-tile.
"""

import sys

for _p in ("/opt/trn_rl_repo",):
    if _p not in sys.path:
        sys.path.append(_p)

import numpy as np
from dataclasses import dataclass

import concourse.bass as bass
import concourse.bacc as bacc
import concourse.mybir as mybir
import concourse.tile as tile
from concourse import masks
from concourse.bass_utils import run_bass_kernel_spmd

try:
    import ml_dtypes

    BF16 = ml_dtypes.bfloat16
except ImportError:  # pragma: no cover
    BF16 = np.float32


def _install_neff_disk_cache():
    """Cache walrus BIR->NEFF compiles on disk (keyed by BIR bytes) so
    repeated runs of the identical graph skip the multi-minute compile."""
    import hashlib, os, tempfile
    from concourse import bass2jax

    if getattr(bass2jax, "_ant_neff_cache_installed", False):
        return
    orig = bass2jax.compile_bir_kernel
    cache_dir = os.path.join(tempfile.gettempdir(), "bass_neff_cache")
    os.makedirs(cache_dir, exist_ok=True)

    def cached(bir_json, tmpdir, neff_name="file.neff"):
        key = hashlib.sha256(bir_json).hexdigest()
        path = os.path.join(cache_dir, key + ".neff")
        dst = os.path.join(tmpdir, neff_name)
        if os.path.exists(path):
            import shutil

            shutil.copy(path, dst)
            return dst
        neff = orig(bir_json, tmpdir, neff_name=neff_name)
        try:
            import shutil

            shutil.copy(neff, path)
        except OSError:
            pass
        return neff

    bass2jax.compile_bir_kernel = cached
    bass2jax._ant_neff_cache_installed = True


_install_neff_disk_cache()

F32 = mybir.dt.float32
BF = mybir.dt.bfloat16
P = 128  # partitions


@dataclass(frozen=True)
class Cfg:
    B: int = 2
    S: int = 2048
    D: int = 1024
    R: int = 128
    N: int = 8
    H: int = 16
    cores: int = 8

    @property
    def G(self):  # cores per batch == head groups
        return self.cores // self.B

    @property
    def S_sl(self):  # sequence slice per core (feature stage)
        return self.S // self.G

    @property
    def COLS(self):  # channel columns per core
        return self.D // self.G

    @property
    def Hpc(self):  # heads per core
        return self.H // self.G

    @property
    def dh(self):
        return self.D // self.H

    @property
    def KD(self):  # k-tiles over D
        return self.D // P

    @property
    def NR(self):
        return self.N * self.R

    @property
    def KNR(self):  # k-tiles over N*R
        return self.NR // P

    @property
    def ST(self):  # s-tiles over full S
        return self.S // P

    @property
    def ST_sl(self):  # s-tiles over S slice
        return self.S_sl // P

    @property
    def CT(self):  # 128-col tiles over COLS
        return (self.COLS + P - 1) // P


FULL = Cfg()


def _ceil_div(a, b):
    return (a + b - 1) // b


def build_nc(cfg: Cfg = FULL, fake_cc: bool = False) -> bacc.Bacc:
    """Build + compile the SPMD graph (identical on every core).

    fake_cc=True replaces collectives with local DMA replication (wrong
    results) so the single-core TimelineSim can cost-model the kernel.
    """
    c = cfg
    assert c.R == P and c.D % P == 0 and c.S_sl % P == 0
    assert P % c.dh == 0 and c.COLS % c.dh == 0 and c.COLS % P == 0

    nc = bacc.Bacc(
        "TRN2",
        target_bir_lowering=False,
        debug=False,
        num_devices=1 if fake_cc else c.cores,
    )

    group0 = list(range(c.G))
    group1 = list(range(c.G, 2 * c.G))
    rgroups = [group0, group1]

    def all_gather(in_ap, out_ap):
        # in_ap has a leading broadcast dim of 1
        if fake_cc:
            nc.sync.dma_start(
                out=out_ap, in_=in_ap.broadcast_to([c.G, *in_ap.shape[1:]])
            )
        else:
            nc.gpsimd.collective_compute(
                "AllGather",
                mybir.AluOpType.bypass,
                replica_groups=rgroups,
                ins=[in_ap[0].opt()],
                outs=[out_ap.opt()],
            )

    def reduce_scatter(in_ap, out_ap):
        if fake_cc:
            nc.sync.dma_start(out=out_ap, in_=in_ap[0])
        else:
            nc.gpsimd.collective_compute(
                "ReduceScatter",
                mybir.AluOpType.add,
                replica_groups=rgroups,
                ins=[in_ap.opt()],
                outs=[out_ap.opt()],
            )

    scale = 1.0 / float(np.sqrt(c.dh))
    DHO = c.dh + 1  # dh + ones column
    n_ch = 2  # feature-weight column chunks
    f_chunk = c.NR // n_ch  # 512
    n_per_ch = f_chunk // c.R  # neurons per chunk

    # ---- DRAM parameters (host-prepped layouts, see shard_inputs) ----
    # xT st-blocked: [P, ST_sl, KD, 128]
    xT = nc.dram_tensor("xT", [P, c.ST_sl, c.KD, P], BF, kind="ExternalInput")
    # feature pools chunk-blocked: [P, n_ch, KD, f_chunk]
    fqk = nc.dram_tensor("fqk", [P, n_ch, c.KD, f_chunk], BF, kind="ExternalInput")
    fv = nc.dram_tensor("fv", [P, n_ch, c.KD, f_chunk], BF, kind="ExternalInput")
    rqk = nc.dram_tensor("rqk", [P, c.KNR, c.COLS], BF, kind="ExternalInput")
    rv = nc.dram_tensor("rv", [P, c.KNR, c.COLS], BF, kind="ExternalInput")
    # W_O row shard: [P, CT, D]  (partition = own d-row within ct block)
    wo = nc.dram_tensor("wo", [P, c.CT, c.D], BF, kind="ExternalInput")
    # combine scalars (feature weights for this core's s-slice), f32
    wcomb = nc.dram_tensor("wcomb", [P, 3, c.ST_sl, c.N], F32, kind="ExternalInput")
    # restore weights, full S, bf16 (broadcast source): [1, 3, N, S]
    wsm = nc.dram_tensor("wsm", [1, 3, c.N, c.S], BF, kind="ExternalInput")
    out_d = nc.dram_tensor("out", [c.S, c.COLS], F32, kind="ExternalOutput")

    # collective bounce buffers (one tensor per chunk: contiguous collectives)
    GRP = 16  # st-tiles per ReduceScatter chunk (single RS: chunked RS races)
    GCH = 2  # local s-tiles per hT gather chunk
    n_gch = c.ST_sl // GCH
    hT_in_c = [
        nc.dram_tensor(f"hT_in{i}", [1, 3, P, GCH * P], BF) for i in range(n_gch)
    ]
    hT_out_c = [
        nc.dram_tensor(f"hT_out{i}", [c.G, 3, P, GCH * P], BF) for i in range(n_gch)
    ]
    n_grp = _ceil_div(c.ST, GRP)
    pr_in_g = [
        nc.dram_tensor(f"pr_in{g}", [c.G, GRP * P, c.COLS], BF) for g in range(n_grp)
    ]
    pr_out_g = [
        nc.dram_tensor(f"pr_out{g}", [GRP * P, c.COLS], BF) for g in range(n_grp)
    ]
    wsm_h1 = nc.dram_tensor("wsm_h1", [3, c.N, c.S // 2], BF)
    wsm_h2 = nc.dram_tensor("wsm_h2", [3, c.N, c.S // 2], BF)

    from contextlib import ExitStack

    with tile.TileContext(nc) as tc, ExitStack() as stack:
        # ------- constants -------
        const_pool = stack.enter_context(tc.tile_pool(name="const", bufs=1))
        ident = const_pool.tile([P, P], BF)
        masks.make_identity(nc, ident[:])
        cmask = const_pool.tile([P, P], BF)
        masks.make_upper_triangular(nc, cmask[:], val=1.0, diag=True)

        # ------- long-lived SBUF residents -------
        res_pool = stack.enter_context(tc.tile_pool(name="residents", bufs=1))
        rqk_sb = res_pool.tile([P, c.KNR, c.COLS], BF)
        rv_sb = res_pool.tile([P, c.KNR, c.COLS], BF)
        wo_sb = res_pool.tile([P, c.CT, c.D], BF)
        wcomb_sb = res_pool.tile([P, 3, c.ST_sl, c.N], F32)

        qT_sb = res_pool.tile([P, c.CT, c.S], BF)
        kT_sb = res_pool.tile([P, c.CT, c.S], BF)
        v_sb = res_pool.tile([P, c.ST, c.Hpc * DHO], BF)

        # scoped pools that span stage A..C; one tile per (t, n, half) so the
        # g-multiply dependencies stay fine-grained
        cstack = ExitStack()
        wrep_pool = cstack.enter_context(tc.tile_pool(name="wrep", bufs=1))
        half = c.S // 2
        wrepT = {
            (t, n, hh): wrep_pool.tile([P, half], BF, name=f"wr{t}_{n}_{hh}")
            for t in range(3)
            for n in range(c.N)
            for hh in range(2)
        }
        hT_pool = cstack.enter_context(tc.tile_pool(name="hTres", bufs=1))
        hT_sb = hT_pool.tile([P, 3, c.G, c.S_sl], BF)

        # ================= Stage A: features on the s-slice =================
        with (
            tc.tile_pool(name="featA", bufs=1) as fpool,
            tc.tile_pool(name="featP", bufs=4, space="PSUM") as fps_pool,
            tc.tile_pool(name="featH", bufs=2) as hpool,
            tc.tile_pool(name="featHT", bufs=2, space="PSUM") as htps_pool,
        ):
            # first-use-ordered resident loads: the critical xT0/fqk0 pair
            # first (each DMA costs 625ns of serial HWDGE before its data)
            half2 = c.S // 2
            xT_sb = fpool.tile([P, c.ST_sl, c.KD, P], BF, tag="xT")
            fqk_sb = fpool.tile([P, n_ch, c.KD, f_chunk], BF, tag="fqk")
            fv_sb = fpool.tile([P, n_ch, c.KD, f_chunk], BF, tag="fv")
            nc.sync.dma_start(out=xT_sb[:, 0], in_=xT[:, 0])
            nc.sync.dma_start(out=fqk_sb[:, 0], in_=fqk[:, 0])
            nc.sync.dma_start(out=wcomb_sb[:], in_=wcomb[:])
            # gate for the first replication wave (early: the wave must clear
            # the pipe before the stage-A-end hT gather traffic)
            nc.sync.dma_start(out=wsm_h1.ap(), in_=wsm.ap()[0, :, :, 0:half2])
            nc.sync.dma_start(out=xT_sb[:, 1], in_=xT[:, 1])
            nc.sync.dma_start(out=fqk_sb[:, 1], in_=fqk[:, 1])
            nc.sync.dma_start(out=xT_sb[:, 2], in_=xT[:, 2])
            nc.sync.dma_start(out=fv_sb[:, 0], in_=fv[:, 0])
            nc.sync.dma_start(out=xT_sb[:, 3], in_=xT[:, 3])
            nc.sync.dma_start(out=fv_sb[:, 1], in_=fv[:, 1])
            # gate for the second replication wave: must be emitted BEFORE
            # its broadcast readers (RAW dep), sits late on the SP FIFO
            nc.sync.dma_start(out=wsm_h2.ap(), in_=wsm.ap()[0, :, :, half2:])
            # Restore-weight replication: DRAM-broadcast DMAs in half-S
            # pieces, gated by DRAM scratch copies placed late on the SP FIFO
            # so the replication waves never clog the input loads.  First
            # halves (needed at C start) gate at ~18us; second halves (needed
            # ~20us into C) gate behind the st=0 hT chain.
            # t=2 (V weights) first halves replicate on the idle Pool engine
            # from separate [1, half] source tiles (partition 0 by
            # construction); their second halves join the C-window DMA wave
            for n in range(c.N):
                wstn = fpool.tile([1, half], BF, tag="wst", bufs=c.N, name=f"wst{n}")
                nc.scalar.dma_start(out=wstn[:], in_=wsm.ap()[0, 2, n : n + 1, 0:half])
                nc.gpsimd.partition_broadcast(wrepT[(2, n, 0)][:], wstn[0:1, :])
            for t in (0, 1):
                for n in range(c.N):
                    nc.scalar.dma_start(
                        out=wrepT[(t, n, 0)][:],
                        in_=wsm_h1.ap()[t, n : n + 1, :].broadcast_to([P, half]),
                    )
            # residents ride the Act DMA queue behind the gated broadcasts
            nc.scalar.dma_start(out=rv_sb[:], in_=rv[:])
            nc.scalar.dma_start(out=rqk_sb[:], in_=rqk[:])
            nc.scalar.dma_start(out=wo_sb[:], in_=wo[:])
            for t in (2, 0, 1):
                for n in range(c.N):
                    nc.scalar.dma_start(
                        out=wrepT[(t, n, 1)][:],
                        in_=wsm_h2.ap()[t, n : n + 1, :].broadcast_to([P, half]),
                    )

            # ones columns of v_sb
            v4 = v_sb[:].rearrange("p st (h x) -> p st h x", x=DHO)
            nc.gpsimd.memset(v4[:, :, :, c.dh : c.dh + 1], 1.0)

            for st in range(c.ST_sl):
                ps_tiles = {}
                for pi, f_sb in ((0, fqk_sb), (1, fv_sb)):
                    for ch in range(n_ch):
                        ps = fps_pool.tile([P, f_chunk], F32, tag="feat")
                        ps_tiles[(pi, ch)] = ps
                        for k in range(c.KD):
                            nc.tensor.matmul(
                                ps[:, :],
                                lhsT=xT_sb[:, st, k, :],
                                rhs=f_sb[:, ch, k, :],
                                start=(k == 0),
                                stop=(k == c.KD - 1),
                            )
                ah_tiles = {}
                for pi in (0, 1):
                    for ch in range(n_ch):
                        ah = hpool.tile([P, f_chunk], BF, tag="ah", bufs=3)
                        nc.scalar.copy(ah[:], ps_tiles[(pi, ch)][:])
                        ah_tiles[(pi, ch)] = ah
                if st % GCH == 0:
                    hT_stage = hpool.tile(
                        [P, 3, GCH * P], BF, tag="hTst", bufs=2, name="hT_stage"
                    )
                for t, pi in ((0, 0), (1, 0), (2, 1)):
                    h_t = hpool.tile([P, c.R], BF, tag="hacc")
                    for n in range(c.N):
                        ah = ah_tiles[(pi, n // n_per_ch)]
                        src = ah[:, c.R * (n % n_per_ch) : c.R * (n % n_per_ch + 1)]
                        if n == 0:
                            nc.vector.tensor_scalar(
                                out=h_t[:],
                                in0=src,
                                scalar1=wcomb_sb[:, t, st, 0:1],
                                scalar2=None,
                                op0=mybir.AluOpType.mult,
                            )
                        else:
                            nc.vector.scalar_tensor_tensor(
                                out=h_t[:],
                                in0=src,
                                scalar=wcomb_sb[:, t, st, n : n + 1],
                                in1=h_t[:],
                                op0=mybir.AluOpType.mult,
                                op1=mybir.AluOpType.add,
                            )
                    htp = htps_pool.tile([P, P], BF, tag="htp")
                    nc.tensor.transpose(htp[:], h_t[:], ident[:])
                    nc.scalar.copy(
                        hT_stage[:, t, P * (st % GCH) : P * (st % GCH + 1)],
                        htp[:, :],
                    )
                if st % GCH == GCH - 1:
                    ch = st // GCH
                    sp = slice(GCH * P * ch, GCH * P * (ch + 1))
                    nc.sync.dma_start(
                        out=hT_in_c[ch].ap()[0].rearrange("t p s -> p t s"),
                        in_=hT_stage[:],
                    )
                    # chunked all-gather: overlaps remaining feature compute
                    all_gather(hT_in_c[ch].ap(), hT_out_c[ch].ap())
                    for t in range(3):
                        nc.sync.dma_start(
                            out=hT_sb[:, t, :, sp],
                            in_=hT_out_c[ch].ap()[:, t].rearrange("g p s -> p g s"),
                        )


        # ============ Stage C: restore projections, per source block ============
        with (
            tc.tile_pool(name="g", bufs=2) as g_pool,
            tc.tile_pool(name="vps", bufs=2, space="PSUM") as vps_pool,
            tc.tile_pool(name="rps", bufs=3, space="PSUM") as rps_pool,
        ):
            for blk in range(c.G):
                bsl = slice(c.S_sl * blk, c.S_sl * (blk + 1))
                g_tiles = {}

                def make_g(t):
                    qsl = slice(c.S_sl * (blk % 2), c.S_sl * (blk % 2 + 1))
                    for n in range(c.N):
                        g_t = g_pool.tile(
                            [P, c.S_sl], BF, tag=f"g{t}_{n}", bufs=1, name=f"g{t}_{n}"
                        )
                        g_tiles[(t, n)] = g_t
                        nc.vector.tensor_mul(
                            g_t[:], hT_sb[:, t, blk, :], wrepT[(t, n, blk // 2)][:, qsl]
                        )

                make_g(2)
                for sti in range(c.ST_sl):
                    st = c.ST_sl * blk + sti
                    vps = vps_pool.tile([P, c.COLS], F32, tag="vps")
                    for n in range(c.KNR):
                        nc.tensor.matmul(
                            vps[:, :],
                            lhsT=g_tiles[(2, n)][:, P * sti : P * (sti + 1)],
                            rhs=rv_sb[:, n, :],
                            start=(n == 0),
                            stop=(n == c.KNR - 1),
                        )
                    nc.scalar.copy(
                        v4[:, st, :, 0 : c.dh],
                        vps[:, :].rearrange("p (h x) -> p h x", x=c.dh),
                    )
                make_g(0)
                make_g(1)
                for t, dst in ((0, qT_sb), (1, kT_sb)):
                    for ct in range(c.CT):
                        pt = min(P, c.COLS - P * ct)
                        rps = rps_pool.tile([P, c.S_sl], F32, tag="rps")
                        for n in range(c.KNR):
                            nc.tensor.matmul(
                                rps[:pt, :],
                                lhsT=rqk_sb[:, n, P * ct : P * ct + pt],
                                rhs=g_tiles[(t, n)][:, :],
                                start=(n == 0),
                                stop=(n == c.KNR - 1),
                            )
                        nc.scalar.copy(dst[:pt, ct, bsl], rps[:pt, :])
        cstack.close()

        # attn tiles live D..F, reusing the wrep/hT space freed above
        dres_pool = stack.enter_context(tc.tile_pool(name="dres", bufs=1))
        attn_sb = dres_pool.tile([P, c.ST, c.Hpc * c.dh], BF)
        attnT_sb = dres_pool.tile([P, c.CT, c.S], BF)

        # ================= Stage D: causal attention per head =================
        # Program order pipelines scores(h+1) ahead of AV(h); Act does only exp
        # in this window so the softmax stream never starves.
        pr_tiles = {}

        def emit_score_tile(h, j):
            ct = (c.dh * h) // P
            off = (c.dh * h) % P
            qlo = P * j
            qn = c.S - qlo
            pj = pr_pool.tile([P, qn], BF, tag=f"p{j}", bufs=2, name=f"p{j}")
            pr_tiles[(h, j)] = pj
            SCH = 1024  # scores chunk (2 PSUM banks); exp whole chunk
            for chk in range(_ceil_div(qn, SCH)):
                lo = qlo + SCH * chk
                hi = min(c.S, lo + SCH)
                sps = sps_pool.tile([P, SCH], F32, tag="sps", bufs=2)
                for sub in range(_ceil_div(hi - lo, 512)):
                    slo = lo + 512 * sub
                    shi = min(hi, slo + 512)
                    nc.tensor.matmul(
                        sps[:, slo - lo : shi - lo],
                        lhsT=kT_sb[off : off + c.dh, ct, qlo : qlo + P],
                        rhs=qT_sb[off : off + c.dh, ct, slo:shi],
                        start=True,
                        stop=True,
                    )
                nc.scalar.activation(
                    pj[:, lo - qlo : hi - qlo],
                    sps[:, 0 : hi - lo],
                    mybir.ActivationFunctionType.Exp,
                    scale=scale,
                )
            # mask the diagonal tile (keep q >= k)
            nc.vector.tensor_mul(pj[:, 0:P], pj[:, 0:P], cmask[:])

        def emit_av_tile(h, j):
            av = av_pool.tile([P, DHO], F32, tag="av", bufs=2)
            for j2 in range(j + 1):
                nc.tensor.matmul(
                    av[:, :],
                    lhsT=pr_tiles[(h, j2)][:, P * (j - j2) : P * (j - j2) + P],
                    rhs=v_sb[:, j2, DHO * h : DHO * (h + 1)],
                    start=(j2 == 0),
                    stop=(j2 == j),
                )
            rec = asm_pool.tile([P, 1], F32, tag="rec")
            nc.vector.reciprocal(rec[:], av[:, c.dh : c.dh + 1])
            nc.vector.tensor_scalar(
                out=attn_sb[:, j, c.dh * h : c.dh * (h + 1)],
                in0=av[:, 0 : c.dh],
                scalar1=rec[:],
                scalar2=None,
                op0=mybir.AluOpType.mult,
            )

        def emit_transposes(ct):
            for st in range(c.ST):
                atp = atps_pool.tile([P, P], BF, tag="atp")
                nc.tensor.transpose(
                    atp[:, :], attn_sb[:, st, P * ct : P * (ct + 1)], ident[:]
                )
                nc.vector.tensor_copy(attnT_sb[:, ct, P * st : P * (st + 1)], atp[:, :])

        with (
            tc.tile_pool(name="probs", bufs=1) as pr_pool,
            tc.tile_pool(name="sps", bufs=1, space="PSUM") as sps_pool,
            tc.tile_pool(name="avps", bufs=1, space="PSUM") as av_pool,
            tc.tile_pool(name="attn_small", bufs=4) as asm_pool,
            tc.tile_pool(name="atps", bufs=2, space="PSUM") as atps_pool,
        ):
            # software-pipelined: scores stay one head ahead of AV, AV tiles
            # interleave between score tiles so PE fills the Act-bound slack
            for j in range(c.ST):
                emit_score_tile(0, j)
            for h in range(c.Hpc):
                for j in range(c.ST):
                    if h + 1 < c.Hpc:
                        emit_score_tile(h + 1, j)
                    emit_av_tile(h, j)
                if (h + 1) * c.dh % P == 0:
                    emit_transposes(((h + 1) * c.dh) // P - 1)

        # ================= Stage F: output projection + ReduceScatter =========
        with (
            tc.tile_pool(name="ops", bufs=4, space="PSUM") as ops_pool,
            tc.tile_pool(name="osb", bufs=4) as osb_pool,
            tc.tile_pool(name="rsb", bufs=4) as rsb_pool,
        ):
            for grp in range(n_grp):
                sts = range(GRP * grp, min(c.ST, GRP * (grp + 1)))
                for st in sts:
                    ops = ops_pool.tile([P, c.D], F32, tag="ops")
                    for ct in range(c.CT):
                        for fc in range(0, c.D, 512):  # one PSUM bank per mm
                            nc.tensor.matmul(
                                ops[:, fc : fc + 512],
                                lhsT=attnT_sb[:, ct, P * st : P * (st + 1)],
                                rhs=wo_sb[:, ct, fc : fc + 512],
                                start=(ct == 0),
                                stop=(ct == c.CT - 1),
                            )
                    # partial out (bf16) -> pr_in[g, st block, :] per dest rank
                    po = osb_pool.tile([P, c.D], BF, tag="po")
                    if st % 2 == 0:
                        nc.scalar.copy(po[:], ops[:, :])
                    else:
                        nc.vector.tensor_copy(po[:], ops[:, :])
                    lo = P * (st - GRP * grp)
                    eng = (nc.gpsimd, nc.scalar, nc.sync, nc.gpsimd)[st % 4]
                    eng.dma_start(
                        out=pr_in_g[grp].ap()[:, lo : lo + P, :].rearrange(
                            "g s x -> s g x"
                        ),
                        in_=po[:].rearrange("p (g x) -> p g x", g=c.G),
                    )
                reduce_scatter(pr_in_g[grp].ap(), pr_out_g[grp].ap())
                # batched readback + f32 convert + final write
                rb = rsb_pool.tile([P, GRP, c.COLS], BF, tag="rb", bufs=2)
                nc.sync.dma_start(
                    out=rb[:],
                    in_=pr_out_g[grp].ap().rearrange("(st p) x -> p st x", p=P),
                )
                ob = rsb_pool.tile([P, GRP, c.COLS], F32, tag="ob", bufs=2)
                if grp % 2 == 0:
                    nc.scalar.copy(ob[:], rb[:])
                else:
                    nc.vector.tensor_copy(ob[:], rb[:])
                nc.scalar.dma_start(
                    out=out_d.ap()[
                        P * GRP * grp : P * GRP * (grp + 1), :
                    ].rearrange("(st p) x -> p st x", p=P),
                    in_=ob[:],
                )

    nc.compile()
    return nc


# ---------------------------------------------------------------------------
# Host-side sharding / gathering
# ---------------------------------------------------------------------------


def shard_inputs(
    inputs: dict,
    cfg: Cfg = FULL,
) -> list[dict]:
    c = cfg
    x = np.asarray(inputs["x"], np.float32)
    fqk_n = np.asarray(inputs["f_qk_neurons"], np.float32)
    fv_n = np.asarray(inputs["f_v_neurons"], np.float32)
    rqk_n = np.asarray(inputs["r_qk_neurons"], np.float32)
    rv_n = np.asarray(inputs["r_v_neurons"], np.float32)
    w_o = np.asarray(inputs["W_O"], np.float32)

    def tile_p(a, kt):  # [D, M] -> [P, kt, M]
        d, m = a.shape
        assert d == kt * P
        return np.ascontiguousarray(a.reshape(kt, P, m).transpose(1, 0, 2))

    # [N, D, R] -> [D, N*R] -> chunk-blocked [P, n_ch, KD, f_chunk]
    n_ch = 2
    f_chunk = c.NR // n_ch

    def feat_layout(fn):
        flat = fn.transpose(1, 0, 2).reshape(c.D, c.NR)  # [D, NR]
        t = tile_p(flat, c.KD)  # [P, KD, NR]
        t = t.reshape(P, c.KD, n_ch, f_chunk).transpose(0, 2, 1, 3)
        return np.ascontiguousarray(t)

    f_qk_l = feat_layout(fqk_n)
    f_v_l = feat_layout(fv_n)
    # [N, R, D] -> [N*R, D]
    r_qk_flat = rqk_n.reshape(c.NR, c.D)
    r_v_flat = rv_n.reshape(c.NR, c.D)

    in_maps = []
    for core in range(c.cores):
        b, g = core // c.G, core % c.G
        sl = slice(c.S_sl * g, c.S_sl * (g + 1))
        cols = slice(c.COLS * g, c.COLS * (g + 1))
        rows = slice(c.COLS * g, c.COLS * (g + 1))

        xTc = x[b].T[:, sl]  # [D, S_sl]
        xt = tile_p(xTc, c.KD)  # [P, KD, S_sl]
        xt = xt.reshape(P, c.KD, c.ST_sl, P).transpose(0, 2, 1, 3)  # [P,ST,KD,128]

        wq = np.asarray(inputs["fqk_weights_Q"], np.float32)[b, sl]  # [S_sl, N]
        wk = np.asarray(inputs["fqk_weights_K"], np.float32)[b, sl]
        wv = np.asarray(inputs["fv_weights"], np.float32)[b, sl]
        wcomb = np.stack([wq, wk, wv], 0)  # [3, S_sl, N]
        wcomb = np.ascontiguousarray(
            wcomb.reshape(3, c.ST_sl, P, c.N).transpose(2, 0, 1, 3)
        )  # [P, 3, ST_sl, N]

        wsm = np.stack(
            [
                np.asarray(inputs["rqk_weights_Q"], np.float32)[b].T,
                np.asarray(inputs["rqk_weights_K"], np.float32)[b].T,
                np.asarray(inputs["rv_weights"], np.float32)[b].T,
            ],
            0,
        ).reshape(1, 3, c.N, c.S)  # [1, 3, N, S]

        # W_O row shard: rows own -> [P, CT, D] (partition = row within ct)
        woR = w_o[rows, :]  # [COLS, D]
        woR = np.ascontiguousarray(woR.reshape(c.CT, P, c.D).transpose(1, 0, 2))

        m = {
            "xT": np.ascontiguousarray(xt).astype(BF16),
            "fqk": f_qk_l.astype(BF16),
            "fv": f_v_l.astype(BF16),
            "rqk": tile_p(r_qk_flat[:, cols], c.KNR).astype(BF16),
            "rv": tile_p(r_v_flat[:, cols], c.KNR).astype(BF16),
            "wo": woR.astype(BF16),
            "wcomb": wcomb.astype(np.float32),
            "wsm": wsm.astype(BF16),
        }
        in_maps.append(m)
    return in_maps


def gather_output(results: list[dict], cfg: Cfg = FULL) -> np.ndarray:
    c = cfg
    out = np.empty((c.B, c.S, c.D), np.float32)
    for core in range(c.cores):
        b, g = core // c.G, core % c.G
        out[b, :, c.COLS * g : c.COLS * (g + 1)] = np.asarray(
            results[core]["out"], np.float32
        )
    return out


_NC_CACHE = {}


def get_nc(cfg: Cfg = FULL) -> bacc.Bacc:
    if cfg not in _NC_CACHE:
        _NC_CACHE[cfg] = build_nc(cfg)
    return _NC_CACHE[cfg]


def kernel(**inputs) -> np.ndarray:
    cfg = FULL
    nc = get_nc(cfg)
    in_maps = shard_inputs(inputs, cfg)
    res = run_bass_kernel_spmd(nc, in_maps, core_ids=list(range(cfg.cores)))
    return gather_output(res.results, cfg)
